# revision 1
# baseline (speedup 1.0000x reference)
"""GNN message-passing (NodeModel) Trainium2 kernel, 8 NeuronCores.

Sharding: edges partitioned by destination node (12500 nodes/core) -> the
segment-sum stays core-local, no collectives. Per core, edges are grouped by
source-node bank (4 banks so gather indices fit int16), sorted by destination,
and laid out in fixed 640-slot blocks per 128 destination nodes so every core
runs the identical program (SPMD, one NEFF).

Math (W1b algebraically deferred out of the edge loop, 16x less edge FLOPs):
  z_e   = P[col_e] + ea_e @ B + b1a,   P = x @ W1a[:32]   (node table, device)
  m_e   = relu(z_e);  S_n = sum_e m_e;  c_n = deg(n)
  agg_n = (S_n / max(c_n,1)) @ W1b + 1[c_n>0] * b1b
  out   = relu([x | agg] @ W2a + b2a) @ W2b + b2b
S and c are accumulated with one-hot matmuls into PSUM (exact; no DMA
scatter-add races). The per-edge bias b1a and the count column ride an
indicator feature through the edge matmul; b1b is folded into
W2a_eff = [W2a; b1b @ W2a[32:]] with the count indicator as a 97th feature.
"""
import numpy as np
import ml_dtypes

N_NODES = 100000
F = 32
HID = 64
NTGT = 32
NCORES = 8

NSH = 12500            # dest nodes per core
NSH_PAD = 12544        # 98 * 128
NBLK = 98              # dest-node blocks (128 nodes) per core
BANKS = 4
BANK_N = 25088         # source nodes per bank (196*128)
TBL_ROWS = BANK_N + 1  # + poison row
POISON = BANK_N
BLK_SLOTS = 640        # slots per (bank, dest-block) = 5 tiles of 128
TPB = 5
NBLK_TOT = BANKS * NBLK            # 392
S_TOT = NBLK_TOT * BLK_SLOTS       # 250880
CHUNK_BLKS = 7
CHUNK = CHUNK_BLKS * BLK_SLOTS     # 4480
CHUNKS_PER_BANK = NBLK // CHUNK_BLKS   # 14
NCHUNK = BANKS * CHUNKS_PER_BANK       # 56
BANK_SLOTS = NBLK * BLK_SLOTS          # 62720
OFFS_NONE = 200.0

_CACHE = {}


def _build_nc():
    import concourse.bass as bass
    import concourse.bacc as bacc
    import concourse.mybir as mybir
    from concourse.tile import TileContext
    from concourse.masks import make_identity
    from bass_rust import add_dep_helper

    f32 = mybir.dt.float32
    bf16 = mybir.dt.bfloat16
    i16 = mybir.dt.int16
    AF = mybir.ActivationFunctionType
    OP = mybir.AluOpType

    nc = bacc.Bacc("TRN2", target_bir_lowering=False, debug=False,
                   num_devices=NCORES)

    xT = nc.dram_tensor("xT", [F, BANKS * BANK_N], f32, kind="ExternalInput")
    x_ownT = nc.dram_tensor("x_ownT", [F, NSH_PAD], f32, kind="ExternalInput")
    A_w = nc.dram_tensor("A_w", [F, HID], f32, kind="ExternalInput")
    Btil_w = nc.dram_tensor("Btil_w", [F + 1, HID + 1], bf16, kind="ExternalInput")
    W1b_w = nc.dram_tensor("W1b_w", [HID, HID], f32, kind="ExternalInput")
    W2a_w = nc.dram_tensor("W2a_w", [F + HID + 1, HID], f32, kind="ExternalInput")
    b2a_w = nc.dram_tensor("b2a_w", [HID, 1], f32, kind="ExternalInput")
    W2b_w = nc.dram_tensor("W2b_w", [HID, NTGT], bf16, kind="ExternalInput")
    b2b_w = nc.dram_tensor("b2b_w", [NTGT, 1], f32, kind="ExternalInput")
    iota_w = nc.dram_tensor("iota_w", [128, 128], bf16, kind="ExternalInput")
    gidx_w = nc.dram_tensor("gidx_w", [128, BANKS * BANK_SLOTS // 16], i16,
                            kind="ExternalInput")
    eaT_w = nc.dram_tensor("eaT_w", [F + 1, S_TOT], bf16, kind="ExternalInput")
    offs_w = nc.dram_tensor("offs_w", [128, S_TOT // 128], bf16,
                            kind="ExternalInput")
    outT = nc.dram_tensor("outT", [NTGT, NSH_PAD], f32, kind="ExternalOutput")

    Ptab = nc.dram_tensor("Ptab", [BANKS, TBL_ROWS, HID], f32, kind="Internal")

    with TileContext(nc) as tc:
        with tc.tile_pool(name="const", bufs=1) as cpool, \
             tc.tile_pool(name="acc", bufs=1) as apool:

            # constants
            A_sb = cpool.tile([F, HID], f32)
            nc.sync.dma_start(out=A_sb[:], in_=A_w[:])
            Btil_sb = cpool.tile([F + 1, HID + 1], bf16)
            nc.sync.dma_start(out=Btil_sb[:], in_=Btil_w[:])
            W1b_sb = cpool.tile([HID, HID], f32)
            nc.sync.dma_start(out=W1b_sb[:], in_=W1b_w[:])
            W2a_sb = cpool.tile([F + HID + 1, HID], f32)
            nc.sync.dma_start(out=W2a_sb[:], in_=W2a_w[:])
            b2a_sb = cpool.tile([HID, 1], f32)
            nc.sync.dma_start(out=b2a_sb[:], in_=b2a_w[:])
            W2b_sb = cpool.tile([HID, NTGT], bf16)
            nc.sync.dma_start(out=W2b_sb[:], in_=W2b_w[:])
            b2b_sb = cpool.tile([NTGT, 1], f32)
            nc.sync.dma_start(out=b2b_sb[:], in_=b2b_w[:])
            iota_sb = cpool.tile([128, 128], bf16)
            nc.sync.dma_start(out=iota_sb[:], in_=iota_w[:])
            offs_sb = cpool.tile([128, S_TOT // 128], bf16)
            nc.sync.dma_start(out=offs_sb[:], in_=offs_w[:])
            idf = cpool.tile([128, 128], f32)
            make_identity(nc, idf[:])

            acc_sb = apool.tile([128, NSH_PAD], f32)
            nc.vector.memset(acc_sb[:], 0.0)

            # ---- Phase 1: P = x @ W1a[:32], node-major, banked table ----
            p_writes = []
            with tc.tile_pool(name="xtp", bufs=2) as xpool, \
                 tc.tile_pool(name="pb", bufs=3) as pbpool, \
                 tc.tile_pool(name="pbp", bufs=3, space="PSUM") as pbppool:
                for b in range(BANKS):
                    for g in range(BANK_N // 3584):     # 7 groups of 3584 nodes
                        xT_sb = xpool.tile([F, 3584], f32, tag="xT")
                        nc.sync.dma_start(
                            out=xT_sb[:],
                            in_=xT[:, b * BANK_N + g * 3584:
                                   b * BANK_N + (g + 1) * 3584])
                        for h in range(7):              # 512-node subgroups
                            ps = pbppool.tile([128, 4 * HID], f32, tag="pbp")
                            for q in range(4):
                                t0 = h * 512 + q * 128
                                nc.tensor.matmul(
                                    ps[:, q * HID:(q + 1) * HID],
                                    xT_sb[:, t0:t0 + 128],
                                    A_sb[:], start=True, stop=True)
                            stg = pbpool.tile([128, 4 * HID], f32, tag="pstg")
                            nc.scalar.activation(stg[:], ps[:], AF.Copy)
                            r0 = g * 3584 + h * 512
                            w = nc.sync.dma_start(
                                out=Ptab[b, r0:r0 + 512, :].rearrange(
                                    "(q p) d -> p q d", p=128),
                                in_=stg[:].rearrange("p (q d) -> p q d", d=HID))
                            p_writes.append(w)
                poi = pbpool.tile([1, HID], f32, tag="poi")
                nc.vector.memset(poi[:], -1e9)
                for b in range(BANKS):
                    w = nc.sync.dma_start(out=Ptab[b, POISON:POISON + 1, :],
                                          in_=poi[:])
                    p_writes.append(w)

            pbar = nc.sync.nop()
            for w in p_writes:
                add_dep_helper(pbar.ins, w.ins, True, "P table done")

            # ---- Phase 2: edge pipeline ----
            # dma_gather is limited to 1024 indices/call (SWDGE ring), so
            # gathers run in 1024-slot calls (61 full + one 256 per bank),
            # decoupled from the 640-slot blocks; per-block adds split at
            # call boundaries (always on 128-slot tile boundaries).
            CALL = 1024
            CALL_TILES = CALL // 128          # 8
            FULL_CALLS = BANK_SLOTS // CALL   # 61
            TAIL = BANK_SLOTS - FULL_CALLS * CALL  # 256
            NCALLS = FULL_CALLS + 1

            with tc.tile_pool(name="gidx", bufs=2) as gxpool, \
                 tc.tile_pool(name="gath", bufs=6) as gpool, \
                 tc.tile_pool(name="ea", bufs=3) as eapool, \
                 tc.tile_pool(name="msg", bufs=3) as mpool, \
                 tc.tile_pool(name="oh", bufs=3) as ohpool, \
                 tc.tile_pool(name="ebp", bufs=3, space="PSUM") as ebppool, \
                 tc.tile_pool(name="accp", bufs=2, space="PSUM") as accppool:

                for b in range(BANKS):
                    gidx_sb = gxpool.tile([128, BANK_SLOTS // 16], i16,
                                          tag="gidx")
                    nc.sync.dma_start(
                        out=gidx_sb[:],
                        in_=gidx_w[:, b * (BANK_SLOTS // 16):
                                   (b + 1) * (BANK_SLOTS // 16)])

                    g_tiles = {}

                    def issue_gather(k):
                        n = CALL if k < FULL_CALLS else TAIL
                        gt = gpool.tile([128, n // 128, HID], f32, tag="gt")
                        gth = nc.gpsimd.dma_gather(
                            gt[:], Ptab[b, :, :],
                            gidx_sb[:, k * (CALL // 16):
                                    k * (CALL // 16) + n // 16],
                            n, n, elem_size=HID, elem_step=HID)
                        add_dep_helper(gth.ins, pbar.ins, True, "gather after P")
                        g_tiles[k] = gt

                    ea_t = None
                    for blk in range(NBLK):
                        t_lo = blk * TPB          # bank-local tile range
                        t_hi = t_lo + TPB
                        k_hi = (t_hi - 1) // CALL_TILES
                        for k in range(len(g_tiles), k_hi + 1):
                            issue_gather(k)

                        if blk % CHUNK_BLKS == 0:
                            ea_t = eapool.tile([F + 1, CHUNK], bf16, tag="eat")
                            slot0 = b * BANK_SLOTS + blk * BLK_SLOTS
                            nc.sync.dma_start(
                                out=ea_t[:],
                                in_=eaT_w[:, slot0:slot0 + CHUNK])
                        cblk = blk % CHUNK_BLKS   # block within ea chunk

                        dblk = blk                # dest col-block (0..97)
                        ebp = ebppool.tile([128, TPB * (HID + 1)], f32,
                                           tag="ebp")
                        for t in range(TPB):
                            s = cblk * BLK_SLOTS + t * 128
                            nc.tensor.matmul(
                                ebp[:, t * (HID + 1):(t + 1) * (HID + 1)],
                                ea_t[:, s:s + 128], Btil_sb[:],
                                start=True, stop=True)
                        ebv = ebp[:].rearrange("p (t e) -> p t e", e=HID + 1)
                        # add gathered P, split at gather-call boundaries
                        t0 = t_lo
                        while t0 < t_hi:
                            k = t0 // CALL_TILES
                            t1 = min(t_hi, (k + 1) * CALL_TILES)
                            nc.vector.tensor_tensor(
                                out=ebv[:, t0 - t_lo:t1 - t_lo, 0:HID],
                                in0=ebv[:, t0 - t_lo:t1 - t_lo, 0:HID],
                                in1=g_tiles[k][:, t0 - k * CALL_TILES:
                                               t1 - k * CALL_TILES, :],
                                op=OP.add)
                            t0 = t1
                        msg = mpool.tile([128, TPB, HID + 1], bf16, tag="msg")
                        nc.scalar.activation(msg[:], ebv[:], AF.Relu)

                        oh = ohpool.tile([128, TPB, 128], bf16, tag="oh")
                        tcol = (b * NBLK + blk) * TPB
                        o_ap = offs_sb[:, tcol:tcol + TPB]
                        o_b = bass.AP(o_ap.tensor, o_ap.offset,
                                      [o_ap.ap[0], o_ap.ap[1], [0, 128]])
                        i_ap = iota_sb[:]
                        i_b = bass.AP(i_ap.tensor, i_ap.offset,
                                      [i_ap.ap[0], [0, TPB], i_ap.ap[1]])
                        nc.vector.tensor_tensor(out=oh[:], in0=o_b, in1=i_b,
                                                op=OP.is_equal)

                        accp = accppool.tile([HID + 1, 128], f32, tag="accp")
                        for t in range(TPB):
                            nc.tensor.matmul(accp[:], msg[:, t, :],
                                             oh[:, t, :],
                                             start=(t == 0), stop=(t == TPB - 1))
                        nc.vector.tensor_tensor(
                            out=acc_sb[0:HID + 1, dblk * 128:(dblk + 1) * 128],
                            in0=acc_sb[0:HID + 1, dblk * 128:(dblk + 1) * 128],
                            in1=accp[:], op=OP.add)

            # ---- Phase 3: node MLP (streamed, 256 nodes per group) ----
            with tc.tile_pool(name="node", bufs=3) as npool, \
                 tc.tile_pool(name="nodep", bufs=1, space="PSUM") as nppool:
                for G in range(NSH_PAD // 256):         # 49 groups
                    # rows: 0:64 Agg^T, 64 ind^T, 65:97 x_own^T
                    rhs2 = npool.tile([F + HID + 1, 256], f32, tag="rhs2")
                    nc.sync.dma_start(out=rhs2[HID + 1:HID + 1 + F, :],
                                      in_=x_ownT[:, G * 256:(G + 1) * 256])
                    for j in range(2):
                        q = 2 * G + j
                        pS = nppool.tile([128, 128], f32, tag="pS")
                        nc.tensor.transpose(
                            out=pS[:], in_=acc_sb[:, q * 128:(q + 1) * 128],
                            identity=idf[:])
                        rec = npool.tile([128, 1], f32, tag="rec")
                        nc.vector.tensor_scalar_max(rec[:], pS[:, HID:HID + 1],
                                                    1.0)
                        nc.vector.reciprocal(rec[:], rec[:])
                        ind = npool.tile([128, 1], f32, tag="ind")
                        nc.vector.tensor_scalar_min(ind[:], pS[:, HID:HID + 1],
                                                    1.0)
                        pA = nppool.tile([128, HID], f32, tag="pA")
                        nc.tensor.matmul(pA[:],
                                         acc_sb[0:HID, q * 128:(q + 1) * 128],
                                         W1b_sb[:], start=True, stop=True)
                        tq = npool.tile([128, 128], f32, tag="tq")
                        nc.vector.tensor_scalar_mul(tq[:, 0:HID], pA[:], rec[:])
                        nc.vector.tensor_copy(out=tq[:, HID:HID + 1], in_=ind[:])
                        nc.vector.memset(tq[:, HID + 1:128], 0.0)
                        pT = nppool.tile([128, 128], f32, tag="pT")
                        nc.tensor.transpose(out=pT[:], in_=tq[:], identity=idf[:])
                        nc.vector.tensor_copy(
                            out=rhs2[0:HID + 1, j * 128:(j + 1) * 128],
                            in_=pT[0:HID + 1, :])
                    ph = nppool.tile([HID, 256], f32, tag="ph")
                    nc.tensor.matmul(ph[:], W2a_sb[:], rhs2[:],
                                     start=True, stop=True)
                    h1 = npool.tile([HID, 256], bf16, tag="h1")
                    nc.scalar.activation(h1[:], ph[:], AF.Relu, bias=b2a_sb[:])
                    po = nppool.tile([NTGT, 256], f32, tag="po")
                    nc.tensor.matmul(po[:], W2b_sb[:], h1[:],
                                     start=True, stop=True)
                    ot = npool.tile([NTGT, 256], f32, tag="ot")
                    nc.scalar.activation(ot[:], po[:], AF.Identity,
                                         bias=b2b_sb[:])
                    nc.sync.dma_start(out=outT[:, G * 256:(G + 1) * 256],
                                      in_=ot[:])

    nc.compile()
    return nc


def _host_prep(x, edge_index, edge_attr, W1a, b1a, W1b, b1b, W2a, b2a,
               W2b, b2b):
    bf = ml_dtypes.bfloat16
    row = np.asarray(edge_index[0], dtype=np.int64)
    col = np.asarray(edge_index[1], dtype=np.int64)
    x = np.asarray(x, dtype=np.float32)
    ea = np.asarray(edge_attr, dtype=np.float32)

    xT_pad = np.zeros((F, BANKS * BANK_N), np.float32)
    xT_pad[:, :N_NODES] = x.T

    Btil = np.zeros((F + 1, HID + 1), np.float32)
    Btil[0:F, 0:HID] = W1a[F:, :]
    Btil[F, 0:HID] = b1a
    Btil[F, HID] = 1.0

    # rhs2 row order: [Agg (64); ind (1); x (32)]
    W2a_eff = np.concatenate(
        [W2a[F:, :], (b1b @ W2a[F:, :]).reshape(1, HID), W2a[:F, :]],
        axis=0).astype(np.float32)

    iota = np.tile(np.arange(128, dtype=np.float32), (128, 1)).astype(bf)

    common = {
        "xT": xT_pad,
        "A_w": np.asarray(W1a[:F, :], np.float32),
        "Btil_w": Btil.astype(bf),
        "W1b_w": np.asarray(W1b, np.float32),
        "W2a_w": W2a_eff,
        "b2a_w": np.asarray(b2a, np.float32).reshape(HID, 1),
        "W2b_w": np.asarray(W2b, np.float32).astype(bf),
        "b2b_w": np.asarray(b2b, np.float32).reshape(NTGT, 1),
        "iota_w": iota,
    }

    shard = row // NSH
    in_maps = []
    for core in range(NCORES):
        sel = np.nonzero(shard == core)[0]
        lrow = (row[sel] - core * NSH).astype(np.int64)
        scol = col[sel]
        bank = scol // BANK_N
        lcol = (scol - bank * BANK_N).astype(np.int64)
        blk = lrow >> 7
        bid = bank * NBLK + blk
        order = np.lexsort((lrow, bid))
        sbid = bid[order]
        cnt = np.bincount(bid, minlength=NBLK_TOT)
        mx = cnt.max()
        assert mx <= BLK_SLOTS, f"block overflow: {mx} > {BLK_SLOTS}"
        starts = np.zeros(NBLK_TOT + 1, np.int64)
        starts[1:] = np.cumsum(cnt)
        within = np.arange(len(order)) - starts[sbid]
        slots = sbid * BLK_SLOTS + within

        gidx = np.full(S_TOT, POISON, np.int32)
        gidx[slots] = lcol[order]
        eaT = np.zeros((F + 1, S_TOT), bf)
        eaT[0:F, slots] = ea[sel][order].T
        eaT[F, slots] = 1.0
        offs = np.full(S_TOT, OFFS_NONE, np.float32)
        offs[slots] = (lrow[order] & 127).astype(np.float32)

        gw = np.empty((128, BANKS * BANK_SLOTS // 16), np.int16)
        for b in range(BANKS):
            blockw = gidx[b * BANK_SLOTS:(b + 1) * BANK_SLOTS].astype(
                np.int16).reshape(BANK_SLOTS // 16, 16).T
            gw[:, b * (BANK_SLOTS // 16):(b + 1) * (BANK_SLOTS // 16)] = \
                np.tile(blockw, (8, 1))

        x_ownT = np.zeros((F, NSH_PAD), np.float32)
        x_ownT[:, :NSH] = x[core * NSH:(core + 1) * NSH].T

        m = dict(common)
        m["x_ownT"] = x_ownT
        m["gidx_w"] = gw
        m["eaT_w"] = eaT
        m["offs_w"] = offs.reshape(S_TOT // 128, 128).T.astype(bf).copy()
        in_maps.append(m)
    return in_maps


def kernel(x, edge_index, edge_attr, u, batch,
           W1a, b1a, W1b, b1b, W2a, b2a, W2b, b2b, _profile=False):
    from concourse import bass_utils

    if "nc" not in _CACHE:
        _CACHE["nc"] = _build_nc()
    nc = _CACHE["nc"]

    in_maps = _host_prep(x, edge_index, edge_attr, W1a, b1a, W1b, b1b,
                         W2a, b2a, W2b, b2b)
    res = bass_utils.run_bass_kernel_spmd(
        nc, in_maps, core_ids=list(range(NCORES)), trace=_profile)
    out = np.empty((N_NODES, NTGT), np.float32)
    for core in range(NCORES):
        out[core * NSH:(core + 1) * NSH] = \
            res.results[core]["outT"][:, :NSH].T
    if _profile:
        _CACHE["last_exec_time_ns"] = res.exec_time_ns
    return out

